# revision 1
# baseline (speedup 1.0000x reference)
"""Trainium2 Bass kernel for 2-layer GAT (nn_GATT_34445637714178).

Strategy: shard destination nodes across 8 cores (segment softmax becomes
core-local), gather source-node feature rows per edge with dma_gather from
a packed table built on-device ([h fp16 x64 | e_src hi/lo | e_dst hi/lo] in
256B bf16 rows), raw-exp softmax (no max subtraction needed at these logit
scales), per-src-chunk layouts (int16 gather index limit) recombined with
dma_scatter_add into a DRAM accumulator.
"""
import sys
import os
import numpy as np


def _ensure_paths():
    for p in ("/opt/trn_rl_repo", "/root/.axon_site/_ro/trn_rl_repo"):
        if p not in sys.path and os.path.isdir(p):
            sys.path.insert(0, p)
    try:
        import concourse.bass  # noqa
    except Exception:
        raise


_ensure_paths()

import concourse.bass as bass
import concourse.bacc as bacc
import concourse.tile as tile
import concourse.mybir as mybir
from concourse import masks
from concourse.bass_utils import run_bass_kernel_spmd

dt = mybir.dt
F32, BF16, FP16, I16 = dt.float32, dt.bfloat16, dt.float16, dt.int16
ALU = mybir.AluOpType
ACTF = mybir.ActivationFunctionType

MASKVAL = -30000.0
NEG_SLOPE = 0.2
NCORES = 8
# gather/scatter per-call index ceiling (Q7 scratch limit safety)
MAX_IDX_CALL = 8192
SEG_COLS = 64            # gather segment column budget (x128 idx)
MAX_GROUPS_CALL = 32     # scatter per-call ceiling (HW-validated <= 6144 idx)
BUCKET_WIDTHS = [1, 2, 3, 4, 5, 6, 7, 8, 10, 12, 14, 16, 20, 24, 32, 48, 64, 96, 128]


from concourse import ap_utils
from concourse.bass import exact_div, round_up_to_multiple


def dma_gather_relaxed(eng, out_ap, in_ap, idxs_ap, num_idxs, num_idxs_reg,
                       elem_size, elem_step, single_packet=False, queue_num=0):
    """dma_gather with the elem_size%%256B assert relaxed to %%32B.

    The 256B multiple is only a transpose-mode requirement; the
    non-transpose Q7 descriptor path handles arbitrary packet sizes
    (HW-validated for 32B and 160B rows)."""
    self = eng
    assert idxs_ap.dtype == mybir.dt.int16
    assert in_ap.dtype == out_ap.dtype
    elem_size_bytes = elem_size * mybir.dt.size(in_ap.dtype)
    assert elem_size_bytes > 0 and elem_size_bytes % 32 == 0
    assert in_ap.space == bass.MemorySpace.DRAM
    assert idxs_ap.space == bass.MemorySpace.SBUF
    assert out_ap.space == bass.MemorySpace.SBUF
    assert ap_utils.ap_is_contiguous(out_ap.ap[1:])
    assert ap_utils.ap_is_contiguous(idxs_ap.ap[1:])
    assert in_ap.ap[-1][1] == out_ap.ap[-1][1] == elem_size
    assert out_ap.ap[0][1] * out_ap.ap[1][1] == round_up_to_multiple(num_idxs, 128)
    assert in_ap.ap[0][0] == elem_step
    stride_bytes = elem_step * mybir.dt.size(in_ap.dtype)
    stride_bytes_256 = exact_div(stride_bytes, 256)
    assert stride_bytes_256 < 256
    _in_ap = self.lower_ap_dma(in_ap, for_custom_bir_dma=True)
    _idxs_ap = self.lower_ap(idxs_ap)
    _out_ap = self.lower_ap(out_ap)
    return self.add_instruction(
        mybir.InstDMAGatherAnt(
            name=self.bass.get_next_instruction_name(),
            ins=[*_in_ap, _idxs_ap,
                 self.lower_val_access(self.to_reg(num_idxs_reg))],
            outs=[_out_ap],
            transpose=False, num_idxs=num_idxs, elem_size=elem_size,
            stride_bytes_256=stride_bytes_256, gen_mode=0,
            single_packet=single_packet, queue_num=queue_num,
            sbuf_tokens_per_rank=0, sbuf_free_dim_per_rank=0,
            sbuf_free_dim_pad_per_rank=0, sbuf_byte_offset=0,
        ))


# ----------------------------------------------------------------------------
# Host-side preprocessing
# ----------------------------------------------------------------------------

def _wrap_idx(idx):
    """[n] int -> [128, n/16] int16 wrapped layout (idx i at [i%16, i//16]),
    replicated across the 8 16-partition groups."""
    n = len(idx)
    assert n % 16 == 0
    a = np.asarray(idx, np.int16).reshape(n // 16, 16).T
    return np.tile(a, (8, 1))


class Layout:
    pass


class _StageDone(Exception):
    def __init__(self, nc):
        self.nc = nc


def preprocess(edge_index, n_nodes, npc, chs, nchunks):
    """Build per-core/per-chunk bucketed slot-major layouts."""
    N = n_nodes
    src = np.concatenate([edge_index[0], np.arange(N, dtype=np.int64)])
    dst = np.concatenate([edge_index[1], np.arange(N, dtype=np.int64)])
    key = dst.astype(np.int64) * N + src.astype(np.int64)
    uniq, counts = np.unique(key, return_counts=True)
    udst = (uniq // N).astype(np.int64)
    usrc = (uniq % N).astype(np.int64)
    lw = np.log(counts.astype(np.float64)).astype(np.float32)

    core_of = udst // npc
    chunk_of = usrc // chs

    dumpbase = ((npc + 127) // 128) * 128
    lay = Layout()
    lay.nchunks = nchunks
    lay.npc = npc
    lay.chs = chs
    # per (c,k): dict with rows
    per_ck = [[None] * nchunks for _ in range(NCORES)]
    for c in range(NCORES):
        selc = core_of == c
        for k in range(nchunks):
            sel = selc & (chunk_of == k)
            ld = (udst[sel] - c * npc).astype(np.int32)
            ls = (usrc[sel] - k * chs).astype(np.int32)
            w = lw[sel]
            # already sorted by (ld, ls)
            nodes, starts, cnts = np.unique(ld, return_index=True, return_counts=True)
            if len(cnts) and cnts.max() > BUCKET_WIDTHS[-1]:
                raise ValueError(f"per-chunk degree {cnts.max()} exceeds bucket cap")
            per_ck[c][k] = dict(nodes=nodes, starts=starts, cnts=cnts, ls=ls, w=w)

    # global bucket schedule per chunk k
    lay.sched = []           # per k: list of (D, ngroups)
    lay.groups = []          # per k: list of (D,) per group (flattened schedule)
    lay.segments = []        # per k: list of (g0, g1, col0, ncols)
    lay.SD = []              # per k: total cols
    lay.Gtot = []
    for k in range(nchunks):
        gb = []
        for D in BUCKET_WIDTHS:
            need = 0
            for c in range(NCORES):
                cnts = per_ck[c][k]["cnts"]
                if D == BUCKET_WIDTHS[0]:
                    nb = int((cnts <= D).sum())
                else:
                    prev = BUCKET_WIDTHS[BUCKET_WIDTHS.index(D) - 1]
                    nb = int(((cnts > prev) & (cnts <= D)).sum())
                need = max(need, (nb + 127) // 128)
            if need:
                gb.append((D, need))
        lay.sched.append(gb)
        groups = []
        for D, ng in gb:
            groups += [D] * ng
        lay.groups.append(groups)
        lay.Gtot.append(len(groups))
        lay.SD.append(int(sum(groups)))
        # segments: greedy pack whole groups, col budget SEG_COLS (or one big group)
        segs = []
        g0, col0, cols = 0, 0, 0
        for gi, D in enumerate(groups):
            if cols and cols + D > SEG_COLS:
                segs.append((g0, gi, col0, cols))
                g0, col0, cols = gi, col0 + cols, 0
            cols += D
        if cols:
            segs.append((g0, len(groups), col0, cols))
        lay.segments.append(segs)
        lay.setdefault_windows = None

    # scatter windows (<=MAX_GROUPS_CALL groups each) per chunk
    lay.windows = []
    for k in range(nchunks):
        G = lay.Gtot[k]
        ws = []
        w0 = 0
        while w0 < G:
            ws.append((w0, min(w0 + MAX_GROUPS_CALL, G)))
            w0 += MAX_GROUPS_CALL
        lay.windows.append(ws)

    # per-core arrays
    lay.eidx = []
    lay.mask = []
    lay.edidx = []
    lay.scidx = []
    maxpads = 0
    for c in range(NCORES):
        e_parts, m_parts, ed_parts, sc_parts = [], [], [], []
        for k in range(nchunks):
            d = per_ck[c][k]
            nodes, starts, cnts, ls, w = d["nodes"], d["starts"], d["cnts"], d["ls"], d["w"]
            order = []  # row assignment per bucket
            Gtot = lay.Gtot[k]
            # assign nodes to buckets
            bidx = np.searchsorted(np.array(BUCKET_WIDTHS), cnts, side="left")
            rows_node = np.full(Gtot * 128, -1, np.int64)   # node id or -1
            rows_start = np.zeros(Gtot * 128, np.int64)
            rows_cnt = np.zeros(Gtot * 128, np.int64)
            gbase = 0
            for D, ng in lay.sched[k]:
                sel = np.array(BUCKET_WIDTHS)[bidx] == D
                nb = nodes[sel]
                sb = starts[sel]
                cb = cnts[sel]
                assert len(nb) <= ng * 128
                pos = gbase * 128 + np.arange(len(nb))
                rows_node[pos] = nb
                rows_start[pos] = sb
                rows_cnt[pos] = cb
                gbase += ng
            # build eidx/mask per group
            ek = np.zeros((lay.SD[k], 128), np.int16)     # [col, p]
            mk = np.full((128, lay.SD[k]), MASKVAL, np.float32)
            col = 0
            for gi, D in enumerate(lay.groups[k]):
                rn = rows_node[gi * 128:(gi + 1) * 128]
                rs = rows_start[gi * 128:(gi + 1) * 128]
                rc = rows_cnt[gi * 128:(gi + 1) * 128]
                jj = np.arange(D)[:, None]                  # [D, 1]
                valid = jj < rc[None, :]                    # [D, 128]
                safe = np.minimum(rs[None, :] + jj, len(ls) - 1 if len(ls) else 0)
                if len(ls):
                    ek[col:col + D, :] = np.where(valid, ls[safe], 0).astype(np.int16)
                    mk[:, col:col + D] = np.where(valid, w[safe], MASKVAL).T
                col += D
            # row-level idx arrays
            edk = np.where(rows_node >= 0, rows_node, 0).astype(np.int16)
            sck = np.empty(Gtot * 128, np.int16)
            padpos = rows_node < 0
            sck[~padpos] = rows_node[~padpos]
            npads = int(padpos.sum())
            maxpads = max(maxpads, npads)
            sck[padpos] = (dumpbase + np.arange(npads)).astype(np.int16)
            e_parts.append(ek.reshape(-1))  # slot-major: pos = col*128 + p
            m_parts.append(mk)
            ed_parts.append(edk)
            sc_parts.append(sck)
        lay.eidx.append(np.concatenate(e_parts))
        lay.mask.append(np.concatenate(m_parts, axis=1))
        lay.edidx.append(np.concatenate(ed_parts))
        lay.scidx.append(np.concatenate(sc_parts))
    lay.padcap = ((maxpads + 128) // 128) * 128
    lay.acc_rows = dumpbase + lay.padcap
    if lay.acc_rows > 32000:
        raise ValueError("accumulator rows exceed int16 scatter range")
    return lay


# ----------------------------------------------------------------------------
# Device kernel builder
# ----------------------------------------------------------------------------

def build_nc(lay, npc, chs, n_nodes, stage=99, timing=False):
    nchunks = lay.nchunks
    NT = (npc + 127) // 128          # node tiles per core
    NPAD = NT * 128
    TOT_E = int(sum(lay.SD))         # total gather cols
    TOT_G = int(sum(lay.Gtot))
    ACC = lay.acc_rows
    W = MAX_GROUPS_CALL
    maxsegc = max(ncols for k in range(nchunks)
                  for (_, _, _, ncols) in lay.segments[k])
    assert maxsegc * 128 <= 12288
    maxwin = max(len(ws) for ws in lay.windows)
    RBT = 8                           # phase-1.5 piece size (node tiles)
    maxD = max(max(lay.groups[k]) for k in range(nchunks))
    PIECE_CAP = max(64, maxD)         # cap C = G*D per compute piece

    nc = bacc.Bacc("TRN2", target_bir_lowering=False, debug=False,
                   num_devices=1 if timing else NCORES)

    x_in = nc.dram_tensor("x", [NPAD, 128], F32, kind="ExternalInput")
    w1_in = nc.dram_tensor("w1", [128, 64], F32, kind="ExternalInput")
    as1_in = nc.dram_tensor("as1", [1, 64], F32, kind="ExternalInput")
    ad1_in = nc.dram_tensor("ad1", [1, 64], F32, kind="ExternalInput")
    b1_in = nc.dram_tensor("b1", [1, 64], F32, kind="ExternalInput")
    w2_in = nc.dram_tensor("w2", [1, 64], F32, kind="ExternalInput")
    p2_in = nc.dram_tensor("p2", [1, 4], F32, kind="ExternalInput")
    eidx_in = nc.dram_tensor("eidx", [128, TOT_E * 8], I16, kind="ExternalInput")
    mask_in = nc.dram_tensor("mask", [128, TOT_E], F32, kind="ExternalInput")
    edidx_in = nc.dram_tensor("edidx", [128, TOT_G * 8], I16, kind="ExternalInput")
    scidx_in = nc.dram_tensor("scidx", [128, TOT_G * 8], I16, kind="ExternalInput")
    out_dram = nc.dram_tensor("out", [128, NT], F32, kind="ExternalOutput")

    t1slice = nc.dram_tensor("t1slice", [NPAD, 128], BF16)
    t1full = nc.dram_tensor("t1full", [NCORES * npc, 128], BF16, addr_space="Shared")
    t2slice = nc.dram_tensor("t2slice", [NPAD, 64], F32)
    t2full = nc.dram_tensor("t2full", [NCORES * npc, 64], F32, addr_space="Shared")
    acc1 = nc.dram_tensor("acc1", [ACC, 128], F32)
    acc2 = nc.dram_tensor("acc2", [ACC, 64], F32)

    with tile.TileContext(nc) as tc:
        with (
            tc.tile_pool(name="const", bufs=1) as cpool,
            tc.tile_pool(name="p0", bufs=2) as p0,
            tc.tile_pool(name="psum", bufs=2, space="PSUM") as psum,
            tc.tile_pool(name="gath", bufs=2) as gpool,
            tc.tile_pool(name="work", bufs=2) as wpool,
            tc.tile_pool(name="edv", bufs=maxwin + 1) as edvpool,
            tc.tile_pool(name="sz", bufs=2) as szpool,
            tc.tile_pool(name="edgp", bufs=2) as edgpool,
            tc.tile_pool(name="idx", bufs=3) as ipool,
        ):
            # ---- constants ----
            ident = cpool.tile([128, 128], F32)
            masks.make_identity(nc, ident[:])
            w1 = cpool.tile([128, 64], F32)
            nc.sync.dma_start(w1[:], w1_in[:])
            as1 = cpool.tile([128, 64], F32)
            nc.sync.dma_start(as1[:], as1_in[:].broadcast_to([128, 64]))
            ad1 = cpool.tile([128, 64], F32)
            nc.sync.dma_start(ad1[:], ad1_in[:].broadcast_to([128, 64]))
            b1r = cpool.tile([128, 64], F32)
            nc.sync.dma_start(b1r[:], b1_in[:].broadcast_to([128, 64]))
            w2r = cpool.tile([128, 64], F32)
            nc.sync.dma_start(w2r[:], w2_in[:].broadcast_to([128, 64]))
            p2r = cpool.tile([128, 4], F32)
            nc.sync.dma_start(p2r[:], p2_in[:].broadcast_to([128, 4]))

            # ---- zero accumulators ----
            zt = cpool.tile([128, 2048], F32)
            nc.vector.memset(zt[:], 0.0)
            for t_dram, width in (() if os.environ.get("K_NOZERO") == "1"
                                  else ((acc1, 128), (acc2, 64))):
                tot = ACC * width
                assert tot % 128 == 0
                per_p = tot // 128
                off = 0
                flat = t_dram[:].rearrange("a b -> (a b)").rearrange(
                    "(p f) -> p f", p=128)
                while off < per_p:
                    n = min(2048, per_p - off)
                    nc.sync.dma_start(flat[:, off:off + n], zt[:, :n])
                    off += n

            # ---- phase 0: h = x @ W1, pack T1 rows ----
            es_all = cpool.tile([128, NT], F32)
            ed_all = cpool.tile([128, NT], F32)
            K_P0MIN = os.environ.get("K_P0MIN") == "1"
            TB = 7
            xin_v = x_in[:].rearrange("(t p) f -> p t f", p=128)
            t1s_v = t1slice[:].rearrange("(t p) f -> p t f", p=128)
            xbt = {}
            rowb = {}
            for t in range(NT):
                b0 = (t // TB) * TB
                if b0 == t:
                    bn = min(TB, NT - b0)
                    xb = p0.tile([128, TB, 128], F32, tag="xb", name="xb%d" % t)
                    nc.scalar.dma_start(xb[:, 0:bn, :], xin_v[:, b0:b0 + bn, :])
                    xbt[b0] = xb
                    rowb[b0] = p0.tile([128, TB, 128], BF16, tag="rowb",
                                       name="rowb%d" % t)
                xt_ = xbt[b0][:, t - b0, :]
                if K_P0MIN:
                    continue
                xT_p = psum.tile([128, 128], F32, tag="xtp")
                nc.tensor.transpose(xT_p[:], xt_, ident[:])
                xT = p0.tile([128, 128], F32, tag="xT")
                nc.vector.tensor_copy(xT[:], xT_p[:])
                h_p = psum.tile([128, 64], F32, tag="hp")
                nc.tensor.matmul(h_p[:], xT[:], w1[:])
                h_t = p0.tile([128, 64], F32, tag="ht")
                nc.vector.tensor_copy(h_t[:], h_p[:])
                scr64 = wpool.tile([128, 64], F32, tag="scr64")
                nc.vector.tensor_tensor(scr64[:], h_t[:], as1[:], op=ALU.mult)
                nc.vector.tensor_reduce(es_all[:, t:t + 1], scr64[:],
                                        axis=mybir.AxisListType.X, op=ALU.add)
                scr64b = wpool.tile([128, 64], F32, tag="scr64b")
                nc.vector.tensor_tensor(scr64b[:], h_t[:], ad1[:], op=ALU.mult)
                nc.vector.tensor_reduce(ed_all[:, t:t + 1], scr64b[:],
                                        axis=mybir.AxisListType.X, op=ALU.add)
                rowt = rowb[b0][:, t - b0, :]
                nc.vector.memset(rowt, 0.0)
                nc.vector.tensor_copy(rowt[:, 0:64].bitcast(FP16), h_t[:])
                for col, vals in ((64, es_all), (66, ed_all)):
                    hi = rowt[:, col:col + 1]
                    nc.vector.tensor_copy(hi, vals[:, t:t + 1])
                    hi32 = p0.tile([128, 1], F32, tag="hi32")
                    nc.vector.tensor_copy(hi32[:], hi)
                    lo32 = p0.tile([128, 1], F32, tag="lo32")
                    nc.vector.tensor_tensor(lo32[:], vals[:, t:t + 1], hi32[:],
                                            op=ALU.subtract)
                    nc.vector.tensor_copy(rowt[:, col + 1:col + 2], lo32[:])
                if t == b0 + min(TB, NT - b0) - 1:
                    bn = min(TB, NT - b0)
                    nc.sync.dma_start(t1s_v[:, b0:b0 + bn, :],
                                      rowb[b0][:, 0:bn, :])

            if os.environ.get("K_AGOFF") == "1":
                pass
            elif timing or os.environ.get("K_NOAG") == "1":
                for r in range(NCORES):
                    nc.sync.dma_start(t1full[r * npc:(r + 1) * npc, :],
                                      t1slice[0:npc, :])
            else:
                nc.gpsimd.collective_compute(
                    "AllGather", ALU.bypass,
                    replica_groups=[list(range(NCORES))],
                    ins=[t1slice[0:npc, :].opt()], outs=[t1full[:].opt()])

            def dbg_dump(ap):
                dbg = wpool.tile([128, NT], F32, tag="dbg", name="dbg")
                nc.sync.dma_start(dbg[:], ap)
                nc.sync.dma_start(out_dram[:], dbg[:])

            if stage <= 1:
                dbg_dump(t1full[0:128 * NT, 0:2].bitcast(F32).rearrange(
                    "(g p) f -> p (g f)", p=128))

            # ---- edge phases ----
            K_NOSCAT = os.environ.get("K_NOSCAT") == "1"
            K_NOCOMP = os.environ.get("K_NOCOMP") == "1"
            K_NOEDG = os.environ.get("K_NOEDG") == "1"
            K_NOGATH = os.environ.get("K_NOGATH") == "1"

            def edge_phase(layer):
                e_off = 0      # global col offset (eidx/mask)
                g_off = 0      # global group offset (edidx/scidx)
                for k in range(nchunks):
                    Gt = lay.Gtot[k]
                    groups = lay.groups[k]
                    windows = lay.windows[k]
                    nSZ = 65 if layer == 1 else 2
                    SDk = lay.SD[k]
                    eix_k = ipool.tile([128, SDk * 8], I16, tag="eixk",
                                       name="eixk_%d_%d" % (layer, k))
                    nc.sync.dma_start(
                        eix_k[:], eidx_in[:, e_off * 8:(e_off + SDk) * 8])
                    msk_k = ipool.tile([128, SDk], F32, tag="mskk",
                                       name="mskk_%d_%d" % (layer, k))
                    nc.sync.dma_start(
                        msk_k[:], mask_in[:, e_off:e_off + SDk])
                    edix_k = ipool.tile([128, Gt * 8], I16, tag="edixk",
                                        name="edixk_%d_%d" % (layer, k))
                    nc.scalar.dma_start(
                        edix_k[:], edidx_in[:, g_off * 8:(g_off + Gt) * 8])
                    scix_k = ipool.tile([128, Gt * 8], I16, tag="scixk",
                                        name="scixk_%d_%d" % (layer, k))
                    nc.scalar.dma_start(
                        scix_k[:], scidx_in[:, g_off * 8:(g_off + Gt) * 8])
                    # per-window ed values + SZ tiles
                    edvs = {}
                    SZs = {}
                    for wi, (gw0, gw1) in enumerate(windows):
                        gn = gw1 - gw0
                        if layer == 1:
                            edg = edgpool.tile([128, W, 16], BF16, tag="edg")
                            srcap = t1slice[:, 64:80]
                            elem, estep = 16, 128
                        else:
                            edg = edgpool.tile([128, W, 8], F32, tag="edg")
                            srcap = t2slice[:, 0:8]
                            elem, estep = 8, 64
                        if not K_NOEDG:
                            dma_gather_relaxed(
                                nc.gpsimd, edg[:, 0:gn, :], srcap,
                                edix_k[:, gw0 * 8:gw1 * 8],
                                num_idxs=gn * 128, num_idxs_reg=gn * 128,
                                elem_size=elem, elem_step=estep,
                                single_packet=False)
                        else:
                            nc.vector.memset(edg[:], 0.0)
                        edv = edvpool.tile([128, W], F32, tag="edv")
                        if layer == 1:
                            nc.vector.tensor_tensor(
                                edv[:, 0:gn], edg[:, 0:gn, 2],
                                edg[:, 0:gn, 3], op=ALU.add)
                        else:
                            nc.vector.tensor_scalar(
                                edv[:, 0:gn], edg[:, 0:gn, 0],
                                scalar1=p2r[:, 1:2], scalar2=None, op0=ALU.mult)
                        edvs[wi] = edv
                        if K_NOCOMP:
                            pass
                        SZs[wi] = szpool.tile(
                            [128, W, nSZ], F32, tag="sz%d" % layer,
                            name="sz_%d_%d_%d" % (layer, k, wi))
                        if K_NOCOMP:
                            nc.vector.memset(SZs[wi][:], 0.0)

                    def scatter_window(wi):
                        gw0, gw1 = windows[wi]
                        gn = gw1 - gw0
                        accap = acc1[:, 0:65] if layer == 1 else acc2[:, 0:2]
                        estep3 = 128 if layer == 1 else 64
                        if not K_NOSCAT:
                            nc.gpsimd.dma_scatter_add(
                                accap, SZs[wi][:, 0:gn, :],
                                scix_k[:, gw0 * 8:gw1 * 8],
                                num_idxs=gn * 128, num_idxs_reg=gn * 128,
                                elem_size=nSZ, elem_step=estep3,
                                single_packet=False)

                    cur_w = 0
                    for (sg0, sg1, col0, ncols) in lay.segments[k]:
                        if layer == 1:
                            gt = gpool.tile([128, maxsegc, 80], BF16, tag="gt")
                            src2 = t1full[k * chs:(k + 1) * chs, 0:80]
                            elem2, estep2 = 80, 128
                        else:
                            gt = gpool.tile([128, maxsegc, 8], F32, tag="gt")
                            src2 = t2full[k * chs:(k + 1) * chs, 0:8]
                            elem2, estep2 = 8, 64
                        if not K_NOGATH:
                            dma_gather_relaxed(
                                nc.gpsimd, gt[:, 0:ncols, :], src2,
                                eix_k[:, col0 * 8:(col0 + ncols) * 8],
                                num_idxs=ncols * 128, num_idxs_reg=ncols * 128,
                                elem_size=elem2, elem_step=estep2,
                                single_packet=False)
                        else:
                            nc.vector.memset(gt[:], 0.25)

                        gi = sg0
                        lcol = 0
                        while (not K_NOCOMP) and gi < sg1:
                            D = groups[gi]
                            wi = gi // W
                            if wi > cur_w:
                                scatter_window(cur_w)
                                cur_w = wi
                            wend = windows[wi][1]
                            gj = gi
                            while (gj < sg1 and gj < wend
                                   and groups[gj] == D
                                   and (gj == gi or (gj - gi + 1) * D <= PIECE_CAP)):
                                gj += 1
                            G = gj - gi
                            C = G * D
                            gsl = gt[:, lcol:lcol + C, :]
                            SZ = SZs[wi]
                            gwi = gi - windows[wi][0]
                            edv = edvs[wi]
                            if layer == 1:
                                es = wpool.tile([128, C], F32, tag="es")
                                nc.vector.tensor_tensor(
                                    es[:], gsl[:, :, 64], gsl[:, :, 65],
                                    op=ALU.add)
                            else:
                                es = wpool.tile([128, C], F32, tag="es")
                                nc.vector.tensor_scalar(
                                    es[:], gsl[:, :, 0], scalar1=p2r[:, 0:1],
                                    scalar2=None, op0=ALU.mult)
                            pre = wpool.tile([128, C], F32, tag="pre")
                            edb = edv[:, gwi:gwi + G].unsqueeze(2).broadcast_to(
                                [128, G, D])
                            nc.vector.tensor_tensor(
                                pre[:].rearrange("p (g d) -> p g d", g=G),
                                es[:].rearrange("p (g d) -> p g d", g=G),
                                edb, op=ALU.add)
                            lk = wpool.tile([128, C], F32, tag="lk")
                            nc.vector.scalar_tensor_tensor(
                                lk[:], pre[:], NEG_SLOPE, pre[:],
                                op0=ALU.mult, op1=ALU.max)
                            lk2 = wpool.tile([128, C], F32, tag="lk2")
                            nc.vector.tensor_tensor(
                                lk2[:], lk[:],
                                msk_k[:, col0 + lcol:col0 + lcol + C],
                                op=ALU.add)
                            w32 = wpool.tile([128, C], F32, tag="w32")
                            nc.scalar.activation(w32[:], lk2[:], ACTF.Exp)
                            if layer == 1:
                                wh = wpool.tile([128, C], FP16, tag="wh")
                                nc.vector.tensor_copy(wh[:], w32[:])
                                M_full = wpool.tile([128, PIECE_CAP, 64], FP16,
                                                    tag="M", name="Mt")
                                M = M_full[:, 0:C, :]
                                h_v = gsl.bitcast(FP16)[:, :, 0:64].rearrange(
                                    "p (g d) f -> p g d f", g=G)
                                w_b = wh[:].rearrange(
                                    "p (g d) -> p g d", g=G).unsqueeze(
                                    3).broadcast_to([128, G, D, 64])
                                nc.vector.tensor_tensor(
                                    M[:].rearrange("p (g d) f -> p g d f", g=G),
                                    h_v, w_b, op=ALU.mult)
                                # unit-stride tree reduction over slots
                                Mv = M[:].rearrange("p (g d) f -> p g d f", g=G)
                                dd = D
                                while dd > 2:
                                    hh = dd // 2
                                    nc.vector.tensor_tensor(
                                        Mv[:, :, 0:hh, :], Mv[:, :, 0:hh, :],
                                        Mv[:, :, dd - hh:dd, :], op=ALU.add)
                                    dd -= hh
                                if dd == 2:
                                    nc.vector.tensor_tensor(
                                        SZ[:, gwi:gwi + G, 0:64],
                                        Mv[:, :, 0, :], Mv[:, :, 1, :],
                                        op=ALU.add)
                                else:
                                    nc.vector.tensor_copy(
                                        SZ[:, gwi:gwi + G, 0:64], Mv[:, :, 0, :])
                                nc.vector.tensor_reduce(
                                    SZ[:, gwi:gwi + G, 64],
                                    w32[:].rearrange("p (g d) -> p g d", g=G),
                                    axis=mybir.AxisListType.X, op=ALU.add)
                            else:
                                gs = gsl[:, :, 0]
                                M2 = wpool.tile([128, C], F32, tag="M2")
                                nc.vector.tensor_tensor(
                                    M2[:], w32[:], gs, op=ALU.mult)
                                nc.vector.tensor_reduce(
                                    SZ[:, gwi:gwi + G, 0],
                                    M2[:].rearrange("p (g d) -> p g d", g=G),
                                    axis=mybir.AxisListType.X, op=ALU.add)
                                nc.vector.tensor_reduce(
                                    SZ[:, gwi:gwi + G, 1],
                                    w32[:].rearrange("p (g d) -> p g d", g=G),
                                    axis=mybir.AxisListType.X, op=ALU.add)
                            gi = gj
                            lcol += C
                    scatter_window(cur_w)
                    e_off += lay.SD[k]
                    g_off += Gt

            if stage >= 2:
                edge_phase(1)
            if stage == 2:
                dbg_dump(acc1[:].rearrange(
                    "(g p) f -> p g f", p=128)[:, 0:NT, 64])

            # ---- phase 1.5: h1, g, T2 (piecewise) ----
            for r0 in (range(0, NT, RBT) if stage >= 3 else []):
                rn = min(RBT, NT - r0)
                rb = wpool.tile([128, RBT, 128], F32, tag="rb")
                rbap = acc1[:].rearrange("(g p) f -> p g f", p=128)[
                    :, r0:r0 + rn, :]
                nc.sync.dma_start(rb[:, 0:rn, :], rbap)
                zs = wpool.tile([128, RBT], F32, tag="zs")
                nc.vector.tensor_scalar(zs[:, 0:rn], rb[:, 0:rn, 64],
                                        scalar1=1e-30, scalar2=None,
                                        op0=ALU.max)
                zr = wpool.tile([128, RBT], F32, tag="zr")
                nc.vector.reciprocal(zr[:, 0:rn], zs[:, 0:rn])
                h1 = wpool.tile([128, RBT, 64], F32, tag="h1")
                nc.vector.tensor_tensor(
                    h1[:, 0:rn, :], rb[:, 0:rn, 0:64],
                    zr[:, 0:rn].unsqueeze(2).broadcast_to([128, rn, 64]),
                    op=ALU.mult)
                nc.vector.tensor_tensor(
                    h1[:, 0:rn, :], h1[:, 0:rn, :],
                    b1r[:].unsqueeze(1).broadcast_to([128, rn, 64]),
                    op=ALU.add)
                nc.scalar.activation(h1[:, 0:rn, :], h1[:, 0:rn, :], ACTF.Relu)
                gsc = wpool.tile([128, RBT, 64], F32, tag="gsc")
                nc.vector.tensor_tensor(
                    gsc[:, 0:rn, :], h1[:, 0:rn, :],
                    w2r[:].unsqueeze(1).broadcast_to([128, rn, 64]),
                    op=ALU.mult)
                t2rows = wpool.tile([128, RBT, 64], F32, tag="t2rows")
                nc.vector.memset(t2rows[:], 0.0)
                nc.vector.tensor_reduce(
                    t2rows[:, 0:rn, 0], gsc[:, 0:rn, :],
                    axis=mybir.AxisListType.X, op=ALU.add)
                nc.sync.dma_start(
                    t2slice[:].rearrange("(g p) f -> p g f", p=128)[
                        :, r0:r0 + rn, :],
                    t2rows[:, 0:rn, :])
            if stage >= 3 and os.environ.get("K_AGOFF") == "1":
                pass
            elif stage >= 3 and timing:
                for r in range(NCORES):
                    nc.sync.dma_start(t2full[r * npc:(r + 1) * npc, :],
                                      t2slice[0:npc, :])
            elif stage >= 3:
                nc.gpsimd.collective_compute(
                    "AllGather", ALU.bypass,
                    replica_groups=[list(range(NCORES))],
                    ins=[t2slice[0:npc, :].opt()], outs=[t2full[:].opt()])
            if stage == 3:
                dbg_dump(t2full[0:128 * NT, 0:1].rearrange(
                    "(g p) f -> p (g f)", p=128))

            if stage >= 4:
                edge_phase(2)
            if stage == 4:
                dbg_dump(acc2[:].rearrange(
                    "(g p) f -> p g f", p=128)[:, 0:NT, 1])

            # ---- phase 2.5: output ----
            for r0 in (range(0, NT, RBT) if stage >= 5 else []):
                rn = min(RBT, NT - r0)
                rb2 = wpool.tile([128, RBT, 64], F32, tag="rb2")
                rb2ap = acc2[:].rearrange("(g p) f -> p g f", p=128)[
                    :, r0:r0 + rn, :]
                nc.sync.dma_start(rb2[:, 0:rn, :], rb2ap)
                zs2 = wpool.tile([128, RBT], F32, tag="zs2")
                nc.vector.tensor_scalar(zs2[:, 0:rn], rb2[:, 0:rn, 1],
                                        scalar1=1e-30, scalar2=None,
                                        op0=ALU.max)
                zr2 = wpool.tile([128, RBT], F32, tag="zr2")
                nc.vector.reciprocal(zr2[:, 0:rn], zs2[:, 0:rn])
                logit = wpool.tile([128, RBT], F32, tag="logit")
                nc.vector.tensor_tensor(logit[:, 0:rn], rb2[:, 0:rn, 0],
                                        zr2[:, 0:rn], op=ALU.mult)
                outt = wpool.tile([128, RBT], F32, tag="outt")
                nc.scalar.activation(outt[:, 0:rn], logit[:, 0:rn],
                                     ACTF.Sigmoid, bias=p2r[:, 2:3])
                nc.sync.dma_start(out_dram[:, r0:r0 + rn], outt[:, 0:rn])

    nc.compile()
    return nc


# ----------------------------------------------------------------------------
# Public entry
# ----------------------------------------------------------------------------

_CACHE = {}


def _kernel_impl(edge_index, x, W1, a_src1, a_dst1, b1, W2, a_src2, a_dst2, b2,
                 run_fn=None):
    N, Din = x.shape
    assert N % NCORES == 0
    npc = N // NCORES
    nchunks = max(1, (N + 32767) // 32768)
    chs = (N + nchunks - 1) // nchunks
    # chunk size must evenly divide? chunks are [k*chs, (k+1)*chs); last may be short
    nchunks = (N + chs - 1) // chs

    ckey = (hash(np.asarray(edge_index).tobytes()), N, Din)
    if ckey in _CACHE:
        lay, nc = _CACHE[ckey]
    else:
        lay = preprocess(np.asarray(edge_index, np.int64), N, npc, chs, nchunks)
        nc = build_nc(lay, npc, chs, N)
        _CACHE.clear()
        _CACHE[ckey] = (lay, nc)

    NT = (npc + 127) // 128
    NPAD = NT * 128
    in_maps = []
    for c in range(NCORES):
        xs = np.zeros((NPAD, 128), np.float32)
        xs[:npc] = np.asarray(x[c * npc:(c + 1) * npc], np.float32)
        in_maps.append({
            "x": xs,
            "w1": np.asarray(W1, np.float32),
            "as1": np.asarray(a_src1, np.float32).reshape(1, 64),
            "ad1": np.asarray(a_dst1, np.float32).reshape(1, 64),
            "b1": np.asarray(b1, np.float32).reshape(1, 64),
            "w2": np.asarray(W2, np.float32).reshape(1, 64),
            "p2": np.array([[float(np.asarray(a_src2).reshape(-1)[0]),
                             float(np.asarray(a_dst2).reshape(-1)[0]),
                             float(np.asarray(b2).reshape(-1)[0]), 0.0]],
                           np.float32),
            "eidx": _wrap_idx(lay.eidx[c]),
            "mask": lay.mask[c],
            "edidx": _wrap_idx(lay.edidx[c]),
            "scidx": _wrap_idx(lay.scidx[c]),
        })

    if run_fn is None:
        res = run_bass_kernel_spmd(nc, in_maps, core_ids=list(range(NCORES)))
        results = res.results
    else:
        results = run_fn(nc, in_maps)

    out = np.empty(N, np.float32)
    for c in range(NCORES):
        o = results[c]["out"]          # [128, NT]
        full = o.T.reshape(-1)          # node n = p + 128*g -> index g*... wait
        # node n at (p=n%128, g=n//128): o[p, g]
        arr = np.empty(NPAD, np.float32)
        g_idx = np.arange(NPAD) // 128
        p_idx = np.arange(NPAD) % 128
        arr = o[p_idx, g_idx]
        out[c * npc:(c + 1) * npc] = arr[:npc]
    return out


def kernel(**inputs):
    return _kernel_impl(**inputs)

